# revision 12
# baseline (speedup 1.0000x reference)
"""LIF (leaky integrate-and-fire) forward kernel for Trainium2, 8 NeuronCores.

Recurrence (per element of [B, N], serial over T):
    v_t = DECAY * w_{t-1} + x_t          (REST = 0, w = post-reset membrane)
    s_t = (v_t > THRESHOLD)
    w_t = v_t * (v_t <= THRESHOLD)

Engine plan (per core, per step tile of [128 partitions, 2048]):
  - DVE: the two fused scalar_tensor_tensor ops of the recurrence (the
    serial chain; ~2.2us each, the kernel's critical path).
  - ScalarE: spike as Sign(v - THR) in fp8 {-1, 0, 1}.
  - PE: packs 8 consecutive steps' signs into one byte-plane via
    accumulating matmuls with stationary weights 2^k * I (fp8) into PSUM;
    ScalarE then converts (psum + 255)/2 -> uint8. Output traffic is 8x
    smaller than storing fp8 spikes, keeping DMA well under the chain.
  - Host decodes bit k of byte-plane g as the spike at t = 8*g + k.

All recurrence arithmetic is fp32 and bitwise-faithful to the reference
ordering. (A byte can only be corrupted if some v_t == THR exactly, which
Sign maps to 0; measure-zero in practice and far inside the 2e-2 budget.)

Sharding: batch dim (128) split 16 rows/core across 8 cores; per-core,
per-step slab is a contiguous 1 MiB block viewed as [128 partitions, 2048].
"""

import numpy as np

import concourse.bacc as bacc
import concourse.mybir as mybir
from concourse.tile import TileContext
from concourse.bass_utils import run_bass_kernel_spmd

T, B, N = 32, 128, 16384
N_CORES = 8
B_SH = B // N_CORES          # 16 batch rows per core
S = B_SH * N                 # 262144 elements per core per time step
P = 128                      # SBUF partitions
F = S // P                   # 2048 free-dim elements
G = T // 8                   # packed byte groups
DECAY = 0.2
THR = 0.3

TRACE = False                # set True (e.g. from test.py) to capture a profile

_BUILT = {}


def _build_nc():
    nc = bacc.Bacc("TRN2", debug=False, num_devices=N_CORES)
    x = nc.dram_tensor("x", [T, S], mybir.dt.float32, kind="ExternalInput").ap()
    y = nc.dram_tensor("y", [G, S], mybir.dt.uint8, kind="ExternalOutput").ap()
    xr = x.rearrange("t (p f) -> t p f", p=P)
    yr = y.rearrange("g (p f) -> g p f", p=P)

    f32 = mybir.dt.float32
    fp8 = mybir.dt.float8e4
    Alu = mybir.AluOpType
    Act = mybir.ActivationFunctionType

    H = F // 2
    with TileContext(nc) as tc:
        with (
            tc.tile_pool(name="state", bufs=1) as state_pool,
            tc.tile_pool(name="xin", bufs=10) as xin_pool,
            tc.tile_pool(name="vtmp", bufs=3) as v_pool,
            tc.tile_pool(name="st", bufs=3) as st_pool,
            tc.tile_pool(name="ob", bufs=2) as ob_pool,
            tc.tile_pool(name="pk", bufs=2, space="PSUM") as psum_pool,
        ):
            negthr = nc.alloc_sbuf_tensor("const_negthr", [P, 1], f32).ap()
            nc.vector.memset(negthr, -THR)

            # Pack weights: wk[k] = 2^k * I in fp8. One Pool iota builds
            # d[p,f] = p - f; tiny DVE tensor_scalars turn it into the eight
            # scaled identities ((d==0) * 2^k). Persistent allocations: all 8
            # must stay alive for the whole kernel.
            wtmp = nc.alloc_sbuf_tensor("wk_iota", [P, 128], f32).ap()
            nc.gpsimd.iota(
                wtmp, pattern=[[-1, 128]], base=0, channel_multiplier=1,
                allow_small_or_imprecise_dtypes=True,
            )
            wks = []
            for k in range(8):
                wk = nc.alloc_sbuf_tensor(f"wk_{k}", [P, 128], fp8).ap()
                nc.vector.tensor_scalar(
                    out=wk, in0=wtmp, scalar1=0.0, scalar2=float(1 << k),
                    op0=Alu.is_equal, op1=Alu.mult,
                )
                wks.append(wk)

            w = state_pool.tile([P, F], f32)

            ps = None
            for t in range(T):
                g, k = divmod(t, 8)
                xt = xin_pool.tile([P, F], f32)
                if t == 0:
                    # quarter the first load so compute starts on 256 KiB
                    for j in range(0, F, 512):
                        nc.sync.dma_start(out=xt[:, j:j + 512], in_=xr[t][:, j:j + 512])
                else:
                    nc.sync.dma_start(out=xt[:], in_=xr[t])

                st = st_pool.tile([P, F], fp8)
                if t == 0:
                    # w_{-1}=0 so v_0 = x_0: read x directly, per quarter
                    for j in range(0, F, 512):
                        nc.vector.scalar_tensor_tensor(
                            out=w[:, j:j + 512], in0=xt[:, j:j + 512], scalar=THR,
                            in1=xt[:, j:j + 512], op0=Alu.is_le, op1=Alu.mult,
                        )
                        nc.scalar.activation(
                            st[:, j:j + 512], xt[:, j:j + 512], Act.Sign, bias=negthr
                        )
                elif t < T - 1:
                    v = v_pool.tile([P, F], f32)
                    # v = w*DECAY + x
                    nc.vector.scalar_tensor_tensor(
                        out=v[:], in0=w[:], scalar=DECAY, in1=xt[:],
                        op0=Alu.mult, op1=Alu.add,
                    )
                    # w = (v<=THR)*v
                    nc.vector.scalar_tensor_tensor(
                        out=w[:], in0=v[:], scalar=THR, in1=v[:],
                        op0=Alu.is_le, op1=Alu.mult,
                    )
                    nc.scalar.activation(st[:], v[:], Act.Sign, bias=negthr)
                else:
                    # last step: w is dead; quarter the work so the tail
                    # (sign -> matmul -> convert -> store) pipelines out
                    v = v_pool.tile([P, F], f32)
                    for j in range(0, F, 512):
                        nc.vector.scalar_tensor_tensor(
                            out=v[:, j:j + 512], in0=w[:, j:j + 512], scalar=DECAY,
                            in1=xt[:, j:j + 512], op0=Alu.mult, op1=Alu.add,
                        )
                        nc.scalar.activation(
                            st[:, j:j + 512], v[:, j:j + 512], Act.Sign, bias=negthr
                        )

                # pack: psum bank j accumulates 2^k * st (identity matmul)
                if k == 0:
                    ps = psum_pool.tile([P, F], f32)
                for j in range(0, F, 512):
                    nc.tensor.matmul(
                        out=ps[:, j:j + 512], lhsT=wks[k][:], rhs=st[:, j:j + 512],
                        start=(k == 0), stop=(k == 7),
                    )
                if k == 7:
                    ob = ob_pool.tile([P, F], mybir.dt.uint8)
                    # (sum_k 2^k sign_k + 255) / 2 -> byte of spike bits
                    if t == T - 1:
                        # stream the last group out per 512-col psum bank
                        for j in range(0, F, 512):
                            nc.scalar.activation(
                                ob[:, j:j + 512], ps[:, j:j + 512], Act.Copy,
                                bias=127.5, scale=0.5,
                            )
                            nc.scalar.dma_start(
                                out=yr[g][:, j:j + 512], in_=ob[:, j:j + 512]
                            )
                    else:
                        nc.scalar.activation(
                            ob[:], ps[:], Act.Copy, bias=127.5, scale=0.5
                        )
                        nc.scalar.dma_start(out=yr[g], in_=ob[:])
    nc.compile()
    return nc


LAST_RESULTS = None


def kernel(tx):
    global LAST_RESULTS
    tx = np.asarray(tx)
    assert tx.shape == (T, B, N) and tx.dtype == np.float32

    if "nc" not in _BUILT:
        _BUILT["nc"] = _build_nc()
    nc = _BUILT["nc"]

    in_maps = [
        {"x": np.ascontiguousarray(tx[:, c * B_SH:(c + 1) * B_SH, :]).reshape(T, S)}
        for c in range(N_CORES)
    ]
    res = run_bass_kernel_spmd(nc, in_maps, core_ids=list(range(N_CORES)), trace=TRACE)
    LAST_RESULTS = res

    out = np.empty((T, B, N), dtype=np.float32)
    for c in range(N_CORES):
        packed = np.asarray(res.results[c]["y"]).reshape(G, B_SH, N, 1)
        bits = np.unpackbits(packed, axis=3, bitorder="little")  # [G, B_SH, N, 8]
        sp = np.moveaxis(bits, 3, 1).reshape(T, B_SH, N)
        out[:, c * B_SH:(c + 1) * B_SH, :] = sp
    return out
